# revision 26
# baseline (speedup 1.0000x reference)
"""Trainium2 Bass kernel for nn_GAT_39427799777563 (GAT message passing).

Math (per item row n, K=32 neighbors, D=100 dims):
    We   = entity_embs * w_r                  # [K, D] elementwise
    e_k  = sum_d We[k, d]                     # neighbor logits
    a_k  = softmax_k(leaky_relu(e_k)) masked by adj
    h'   = sum_k a_k * We[k, :]               # weighted neighbor sum
    x    = h' @ W_out.T + b_out + item_embs

v2 design (vs the fp32 v1 at ~307us):
  * fp16 everywhere on the wire: ent/wr are loaded as one interleaved
    fp16 buffer (halves HBM traffic, the roofline term). fp16 (not bf16)
    because exp() amplifies e-sum rounding ~10x: bf16 inputs alone give
    1.7e-2 absmax-rel (gate 2e-2); fp16 gives 7.3e-3 (simulated).
  * k-innermost layout [row, j, d, k]: every elementwise op and tree-add
    has a packed 2-byte innermost AP dim, which turns on the DVE 2x mode
    (tensor_tensor 2x_1p). Crucially the attention-broadcast multiply
    q = We * a (broadcast over d) keeps k innermost-contiguous, so it
    runs 2x too - impossible in d-innermost layout (stride-0 innermost).
  * reductions as fp16 tree-adds (tensor_reduce never gets the 2x mode;
    tensor_tensor does): e-sum over d and h'-sum over k each cost ~half
    a strided reduce, with fp32 final level.
  * attention normalized BEFORE weighting (a = p/sum(p) in [0,1], fp16-
    safe; raw exp(e) ~ 1e17 is not), so the matmul epilogue is a plain
    residual add.
  * adj mask folded into the host packing: masked/padding slots get a
    poison pair ent[d0] = -244, wr[d0] = 244 (product -59536, exp -> 0
    exactly in fp32), so no adj tensor is loaded and no mask multiply.
  * engine balance per 256-row pair (DMA floor ~4.8us): DVE does the two
    big 2x multiplies + most tree levels (~5us); ACT takes the e-sums of
    the last few k's via activation(Copy, accum_out) plus exp and the
    PSUM->SBUF copies (~4.5us); GPSIMD takes the h'-tree first level and
    the residual epilogue (~3.8us); PE does transpose + the 100x100
    linear in fp16.

Sparsity packing as v1: active k's packed front per row, rows sorted by
count, 256-row pairs striped across the 8 SPMD cores, per-pair-slot K =
max over its 8 cores. Rows un-permuted on host after the gather.
"""

from contextlib import ExitStack

import numpy as np

import concourse.bass as bass
import concourse.bacc as bacc
import concourse.mybir as mybir
import concourse.tile as tile

F32 = mybir.dt.float32
F16 = mybir.dt.float16
ALPHA = 0.2
POISON = 244.0  # ent=-244, wr=+244 -> We=-59536 (fp16-exact), exp -> 0

N, K, D = 40000, 32, 100
N_CORES = 8
P = 128            # rows per tile == SBUF partitions
J = 2              # tiles per pair
STORE_CHUNK = 8    # tiles per output store
_N_TILES_FULL = 40  # 8 cores * 40 tiles * 128 rows = 40960 >= 40000

import os as _os
# engine-balance knobs. GPSIMD measured ~4x slower than its cost model on
# real HW (300us vs 215us all-DVE), so the Pool fracs default to 0.
Q_ACT = int(_os.environ.get("GAT_Q_ACT", "7"))        # k's of q-mul on ACT (Copy+scale)
E_L1_POOL = float(_os.environ.get("GAT_E_L1_POOL", "0"))  # frac of e-tree L1 on GPSIMD
H_L1_POOL = float(_os.environ.get("GAT_H_L1_POOL", "0"))  # frac of h-tree L1 on GPSIMD
BUFS = int(_os.environ.get("GAT_BUFS", "3"))          # compute pool buffering


def _tree_steps(s):
    """Halving steps for an in-place prefix tree-sum of s elements:
    out[0:h] += in[keep:s], leaving keep = s - h live. Ends at s == 2."""
    steps = []
    while s > 2:
        h = s // 2
        steps.append((h, s - h, s))
        s = s - h
    return steps


def _tree_steps_even(s):
    """Like _tree_steps but h/keep stay even: the DVE 2x (fp16) mode
    requires 4B-aligned accesses, so fp16 slices along the innermost k
    axis must start at even element offsets."""
    assert s % 2 == 0
    steps = []
    while s > 2:
        h = (s // 2) & ~1
        steps.append((h, s - h, s))
        s = s - h
    return steps


def build(n_tiles: int, repeats: int = 1, mode: str = "full", klist=None):
    if klist is None:
        klist = [K] * (n_tiles // J)
    assert len(klist) == n_tiles // J
    sumkf = sum(2 * J * kp * D for kp in klist)  # ent+wr combined elems/row-p

    nc = bacc.Bacc("TRN2", target_bir_lowering=False, debug=False,
                   num_devices=N_CORES)

    cw_d = nc.dram_tensor("cw", [P * sumkf], F16, kind="ExternalInput")
    itemb_d = nc.dram_tensor("itemb", [P, n_tiles * D], F16, kind="ExternalInput")
    wt_d = nc.dram_tensor("wt", [D, D], F16, kind="ExternalInput")   # W_out.T
    ident_d = nc.dram_tensor("ident", [P, P], F16, kind="ExternalInput")
    out_d = nc.dram_tensor("out", [P, n_tiles * D], F16, kind="ExternalOutput")

    kmax = max(klist)

    with tile.TileContext(nc) as tc, ExitStack() as ctx:
        # bufs sized to each tile's pipeline span (stage of production
        # through stage of last use, see body_pairs)
        const = ctx.enter_context(tc.tile_pool(name="const", bufs=1))
        cwp = ctx.enter_context(tc.tile_pool(name="cwp", bufs=BUFS))
        wep = ctx.enter_context(tc.tile_pool(name="wep", bufs=BUFS + 1))
        qp = ctx.enter_context(tc.tile_pool(name="qp", bufs=BUFS))
        esp = ctx.enter_context(tc.tile_pool(name="esp", bufs=2))
        small = ctx.enter_context(tc.tile_pool(name="small", bufs=BUFS))
        psum = ctx.enter_context(tc.tile_pool(name="psum", bufs=4, space="PSUM"))

        itemb = const.tile([P, n_tiles * D], F16)
        wt = const.tile([D, D], F16)
        ident = const.tile([P, P], F16)
        out_all = const.tile([P, n_tiles * D], F16)
        nc.sync.dma_start(itemb[:], itemb_d[:])
        nc.sync.dma_start(wt[:], wt_d[:])
        nc.sync.dma_start(ident[:], ident_d[:])

        def tile_loop():
            body_pairs(nc, n_tiles, klist, kmax, cw_d, out_d, itemb, wt,
                       ident, out_all, cwp, wep, qp, esp, small, psum, mode)

        if repeats > 1:
            with tc.For_i(0, repeats, 1):
                tile_loop()
        else:
            tile_loop()

    nc.compile()
    return nc


def body_pairs(nc, n_tiles, klist, kmax, cw_d, out_d, itemb, wt, ident,
               out_all, cwp, wep, qp, esp, small, psum, mode):
    """Software-pipelined pair loop.

    The kernel is latency-bound, not throughput-bound: each pair's chain
    (DMA -> mul -> e-tree -> softmax -> q -> h-tree -> PE -> out) is ~10us
    of serially-dependent work, and measurements show the tile scheduler
    does NOT overlap consecutive pairs on its own (engines run their
    streams in order, so a mid-pair stall blocks the next pair's ops).
    Emitting the stages interleaved - at tick t, stage s handles pair
    t - s, deepest stage first - keeps ~7 pairs in flight so every
    engine always has ready work from SOME pair.
    """
    AF = mybir.ActivationFunctionType
    AL = mybir.AluOpType
    n_pairs = n_tiles // J

    offs = []
    off = 0
    for kp in klist:
        offs.append(off)
        off += 2 * J * P * kp * D

    st = [None] * n_pairs

    def s0_load(pg):
        kp = klist[pg]
        kf = kp * D
        blk = 2 * J * P * kf
        # one interleaved DMA: [:, :J*kf] = ent, [:, J*kf:] = wr
        cw = cwp.tile([P, 2 * J * kmax * D], F16, tag="cw")
        nc.sync.dma_start(
            cw[:, :2 * J * kf].rearrange("p (t j f) -> p t j f", t=2, j=J),
            cw_d[offs[pg]:offs[pg] + blk].rearrange(
                "(t j p f) -> p t j f", t=2, j=J, p=P))
        st[pg] = {"cw": cw, "kp": kp, "kf": kf}

    def s1_wemul(pg):
        d = st[pg]
        kp, kf, cw = d["kp"], d["kf"], d["cw"]
        we = wep.tile([P, J * kmax * D], F16, tag="we")
        nc.vector.tensor_mul(we[:, :J * kf], cw[:, :J * kf],
                             cw[:, J * kf:2 * J * kf])
        d["we"] = we
        d["we4"] = we[:, :J * kf].rearrange("p (j d k) -> p j d k",
                                            j=J, k=kp)

    def s2_elogits(pg):
        d = st[pg]
        kp, kf, we = d["kp"], d["kf"], d["we"]
        # e_{j,k} = sum_d We[j, d, k]: fp16 tree over d. All slices are
        # d-ranges of the (j, d, k) layout = contiguous runs, so the APs
        # merge and the DVE runs at full fp16 rate (short innermost runs
        # cost ~7 cycles of AP-walk per run and must be avoided).
        wej = we[:, :J * kf].rearrange("p (j f) -> p j f", j=J)
        e = small.tile([P, J * kmax], F32, tag="e")
        es = esp.tile([P, J * 50 * kmax], F16, tag="es")
        ese = es[:, :J * 50 * kp].rearrange("p (j f) -> p j f", j=J)
        nc.vector.tensor_add(ese, wej[:, :, 0:50 * kp],
                             wej[:, :, 50 * kp:100 * kp])
        for h, keep, s in _tree_steps(50):
            nc.vector.tensor_add(ese[:, :, 0:h * kp],
                                 ese[:, :, 0:h * kp],
                                 ese[:, :, keep * kp:s * kp])
        nc.vector.tensor_add(
            e[:, :J * kp].rearrange("p (j k) -> p j k", j=J),
            ese[:, :, 0:kp], ese[:, :, kp:2 * kp])
        # leaky relu (DVE), exp (ACT) - stage ends at the ACT handoff so
        # the exp has a full tick before s3 consumes it
        elr = small.tile([P, J * kmax], F32, tag="elr")
        nc.vector.scalar_tensor_tensor(elr[:, :J * kp], e[:, :J * kp],
                                       ALPHA, e[:, :J * kp],
                                       op0=AL.mult, op1=AL.max)
        p = small.tile([P, J * kmax], F32, tag="p")
        nc.scalar.activation(p[:, :J * kp], elr[:, :J * kp], AF.Exp)
        d["p"] = p

    def s3_attq(pg):
        d = st[pg]
        kp, kf, p, we4 = d["kp"], d["kf"], d["p"], d["we4"]
        # normalize attention before weighting: a = p / sum_k p (in [0,1],
        # fp16-safe; raw exp(e) ~ 1e17 is not)
        sumexp = small.tile([P, J], F32, tag="sumexp")
        nc.vector.tensor_reduce(
            sumexp[:], p[:, :J * kp].rearrange("p (j k) -> p j k", j=J),
            axis=mybir.AxisListType.X, op=AL.add)
        rs = small.tile([P, J], F32, tag="rs")
        nc.vector.reciprocal(rs[:], sumexp[:])
        ph = small.tile([P, J * kmax], F16, tag="ph")
        for j in range(J):
            jsl = slice(j * kp, (j + 1) * kp)
            nc.vector.scalar_tensor_tensor(ph[:, jsl], p[:, jsl],
                                           rs[:, j:j + 1], p[:, jsl],
                                           op0=AL.mult, op1=AL.bypass)
        # q = We * a  (DVE fp16 2x: broadcast over d keeps k innermost)
        q = qp.tile([P, J * kmax * D], F16, tag="q")
        q4 = q[:, :J * kf].rearrange("p (j d k) -> p j d k", j=J, k=kp)
        ph4 = (ph[:, :J * kp].rearrange("p (j k) -> p j k", j=J)
               .unsqueeze(2).broadcast_to([P, J, D, kp]))
        nc.vector.tensor_mul(q4, we4, ph4)
        d["q4"] = q4

    def s4_htrans(pg):
        d = st[pg]
        kp, q4 = d["kp"], d["q4"]
        # h'^T[j] = sum_k q[:, j, :, k]^T, accumulated exactly in fp32
        # PSUM by one PE matmul per k (lhsT = strided k-slice of q,
        # rhs = identity -> transpose; start=False accumulates). This
        # replaces the DVE k-tree, whose short innermost runs made it
        # ~5x slower than its element count suggests.
        d["htp"] = []
        for j in range(J):
            ht_ps = psum.tile([D, P], F32, tag="htp")
            for k in range(kp):
                nc.tensor.matmul(ht_ps[:], q4[:, j:j + 1, :, k:k + 1],
                                 ident[:], start=(k == 0),
                                 stop=(k == kp - 1))
            d["htp"].append(ht_ps)

    def s5_pe(pg):
        d = st[pg]
        d["xps"] = []
        for j in range(J):
            t = pg * J + j
            # h'^T PSUM -> SBUF fp16 (ACT)
            ht = small.tile([D, P], F16, tag="ht")
            nc.scalar.copy(ht[:], d["htp"][j][:])
            # x = h' @ W_out.T + (item + b): the residual rides the PSUM
            # accumulation as a second matmul (ident lhsT copies itemb in)
            x_ps = psum.tile([P, D], F32, tag="xps")
            nc.tensor.matmul(x_ps[:], ht[:], wt[:], start=True, stop=False)
            nc.tensor.matmul(x_ps[:], ident[:],
                             itemb[:, t * D:(t + 1) * D],
                             start=False, stop=True)
            d["xps"].append(x_ps)

    def s6_out(pg):
        d = st[pg]
        for j in range(J):
            t = pg * J + j
            # PSUM -> SBUF fp16 (ACT)
            nc.scalar.copy(out_all[:, t * D:(t + 1) * D], d["xps"][j])
        if (pg + 1) % (STORE_CHUNK // J) == 0:
            csl = slice((pg + 1 - STORE_CHUNK // J) * J * D,
                        (pg + 1) * J * D)
            nc.sync.dma_start(out_d[:, csl], out_all[:, csl])
        st[pg] = None

    if mode == "dma":
        for pg in range(n_pairs):
            s0_load(pg)
            kf = st[pg]["kf"]
            for j in range(J):
                t = pg * J + j
                nc.vector.tensor_copy(out_all[:, t * D:(t + 1) * D],
                                      st[pg]["cw"][:, j * kf:j * kf + D])
            if (pg + 1) % (STORE_CHUNK // J) == 0:
                csl = slice((pg + 1 - STORE_CHUNK // J) * J * D,
                            (pg + 1) * J * D)
                nc.sync.dma_start(out_d[:, csl], out_all[:, csl])
    else:
        stages = [s0_load, s1_wemul, s2_elogits, s3_attq, s4_htrans,
                  s5_pe, s6_out]
        NS = len(stages)
        for t in range(n_pairs + NS - 1):
            for s in reversed(range(NS)):
                pg = t - s
                if 0 <= pg < n_pairs:
                    stages[s](pg)

    rem = n_pairs % (STORE_CHUNK // J)
    if rem:
        csl = slice((n_pairs - rem) * J * D, n_pairs * J * D)
        nc.sync.dma_start(out_d[:, csl], out_all[:, csl])


def _shard_host(item_embs, entity_embs, w_r, adj, W_out, b_out, n_tiles):
    """Sort rows by active-neighbor count, pack active k's first, poison the
    masked tail slots, transpose each row to [D, kp] (k innermost), fp16,
    and interleave ent|wr into one per-core buffer. Pairs striped across
    cores as v1. Returns (in_maps, klist, order)."""
    rows = n_tiles * P
    n_pad = N_CORES * rows
    n_pairs = n_tiles // J

    ent = np.asarray(entity_embs, np.float32).reshape(N, K, D)
    wr = np.asarray(w_r, np.float32).reshape(N, K, D)
    adjf = np.asarray(adj).astype(np.float32)
    itemb = np.asarray(item_embs, np.float32) + np.asarray(b_out, np.float32)

    pad = n_pad - N
    ent = np.pad(ent, ((0, pad), (0, 0), (0, 0)))
    wr = np.pad(wr, ((0, pad), (0, 0), (0, 0)))
    # padding rows: one active zero neighbor -> e=0, sumexp=1 (count 1
    # sorts them to the sparse end); their output rows are discarded.
    adjp = np.pad(adjf, ((0, pad), (0, 0)))
    adjp[N:, 0] = 1.0
    itemb = np.pad(itemb, ((0, pad), (0, 0)))

    counts = adjp.sum(1).astype(np.int64)
    order = np.argsort(counts, kind="stable")

    # round packed K up to even: the DVE fp16 2x mode needs 4B alignment,
    # so odd kp would misalign every (j, d) row and all k-axis tree slices
    pair_k = counts[order].reshape(-1, J * P).max(1)
    klist = []
    for j in range(n_pairs):
        kp = max(2, int(pair_k[8 * j: 8 * j + 8].max()))
        klist.append(kp + (kp & 1))

    ai_full = np.argsort(1.0 - adjp, axis=1, kind="stable")  # active first

    wt = np.ascontiguousarray(np.asarray(W_out, np.float32).T).astype(np.float16)
    ident = np.eye(P, dtype=np.float16)

    in_maps = []
    for c in range(N_CORES):
        cw_parts = []
        it_sw = np.empty((P, n_tiles * D), np.float16)
        for j in range(n_pairs):
            g = 8 * j + c
            rsel = order[g * J * P:(g + 1) * J * P]
            kp = klist[j]
            ai = ai_full[rsel, :kp]
            cnt = counts[rsel]                               # [256]
            eg = np.take_along_axis(ent[rsel], ai[:, :, None], 1)  # [256,kp,D]
            wg = np.take_along_axis(wr[rsel], ai[:, :, None], 1)
            # poison masked tail slots: We[d0] = -POISON^2, rest 0 -> exp=0
            mask = np.arange(kp)[None, :] >= cnt[:, None]    # [256, kp]
            eg[mask] = 0.0
            wg[mask] = 0.0
            eg[:, :, 0][mask] = -POISON
            wg[:, :, 0][mask] = POISON
            # k-innermost: [256, kp, D] -> [256, D, kp]; fp16
            eg = eg.transpose(0, 2, 1).astype(np.float16)
            wg = wg.transpose(0, 2, 1).astype(np.float16)
            cw_parts.append(eg.ravel())
            cw_parts.append(wg.ravel())
            it = itemb[rsel].reshape(J, P, D).transpose(1, 0, 2)
            it_sw[:, j * J * D:(j + 1) * J * D] = \
                it.reshape(P, J * D).astype(np.float16)
        in_maps.append({
            "cw": np.concatenate(cw_parts),
            "itemb": it_sw,
            "wt": wt,
            "ident": ident,
        })
    return in_maps, klist, order


def _unshard_host(results, n_tiles, order):
    n_pairs = n_tiles // J
    res_sorted = np.empty((N_CORES * n_tiles * P, D), np.float32)
    for c in range(N_CORES):
        o = results[c]["out"].astype(np.float32)  # [P, n_tiles * D] fp16
        for j in range(n_pairs):
            g = 8 * j + c
            blk = (o[:, j * J * D:(j + 1) * J * D]
                   .reshape(P, J, D).transpose(1, 0, 2).reshape(J * P, D))
            res_sorted[g * J * P:(g + 1) * J * P] = blk
    out = np.empty_like(res_sorted)
    out[order] = res_sorted
    return out[:N]


def kernel(item_embs, entity_embs, w_r, adj, W_out, b_out):
    from concourse.bass_utils import run_bass_kernel_spmd

    in_maps, klist, order = _shard_host(item_embs, entity_embs, w_r, adj,
                                        W_out, b_out, _N_TILES_FULL)
    nc = build(_N_TILES_FULL, klist=klist)
    res = run_bass_kernel_spmd(nc, in_maps, core_ids=list(range(N_CORES)))
    return _unshard_host(res.results, _N_TILES_FULL, order).astype(np.float32)


# revision 30
# speedup vs baseline: 1.1325x; 1.1325x over previous
"""Trainium2 Bass kernel for nn_GAT_39427799777563 (GAT message passing).

Math (per item row n, K=32 neighbors, D=100 dims):
    We   = entity_embs * w_r                  # [K, D] elementwise
    e_k  = sum_d We[k, d]                     # neighbor logits
    a_k  = softmax_k(leaky_relu(e_k)) masked by adj
    h'   = sum_k a_k * We[k, :]               # weighted neighbor sum
    x    = h' @ W_out.T + b_out + item_embs

v2 design (vs the fp32 v1 at ~307us):
  * fp16 everywhere on the wire: ent/wr are loaded as one interleaved
    fp16 buffer (halves HBM traffic, the roofline term). fp16 (not bf16)
    because exp() amplifies e-sum rounding ~10x: bf16 inputs alone give
    1.7e-2 absmax-rel (gate 2e-2); fp16 gives 7.3e-3 (simulated).
  * k-innermost layout [row, j, d, k]: every elementwise op and tree-add
    has a packed 2-byte innermost AP dim, which turns on the DVE 2x mode
    (tensor_tensor 2x_1p). Crucially the attention-broadcast multiply
    q = We * a (broadcast over d) keeps k innermost-contiguous, so it
    runs 2x too - impossible in d-innermost layout (stride-0 innermost).
  * reductions as fp16 tree-adds (tensor_reduce never gets the 2x mode;
    tensor_tensor does): e-sum over d and h'-sum over k each cost ~half
    a strided reduce, with fp32 final level.
  * attention normalized BEFORE weighting (a = p/sum(p) in [0,1], fp16-
    safe; raw exp(e) ~ 1e17 is not), so the matmul epilogue is a plain
    residual add.
  * adj mask folded into the host packing: masked/padding slots get a
    poison pair ent[d0] = -244, wr[d0] = 244 (product -59536, exp -> 0
    exactly in fp32), so no adj tensor is loaded and no mask multiply.
  * engine balance per 256-row pair (DMA floor ~4.8us): DVE does the two
    big 2x multiplies + most tree levels (~5us); ACT takes the e-sums of
    the last few k's via activation(Copy, accum_out) plus exp and the
    PSUM->SBUF copies (~4.5us); GPSIMD takes the h'-tree first level and
    the residual epilogue (~3.8us); PE does transpose + the 100x100
    linear in fp16.

Sparsity packing as v1: active k's packed front per row, rows sorted by
count, 256-row pairs striped across the 8 SPMD cores, per-pair-slot K =
max over its 8 cores. Rows un-permuted on host after the gather.
"""

from contextlib import ExitStack

import numpy as np

import concourse.bass as bass
import concourse.bacc as bacc
import concourse.mybir as mybir
import concourse.tile as tile

F32 = mybir.dt.float32
F16 = mybir.dt.float16
ALPHA = 0.2
POISON = 244.0  # ent=-244, wr=+244 -> We=-59536 (fp16-exact), exp -> 0

N, K, D = 40000, 32, 100
N_CORES = 8
P = 128            # rows per tile == SBUF partitions
J = 2              # tiles per pair
STORE_CHUNK = 8    # tiles per output store
_N_TILES_FULL = 40  # 8 cores * 40 tiles * 128 rows = 40960 >= 40000

import os as _os
# engine-balance knobs. GPSIMD measured ~4x slower than its cost model on
# real HW (300us vs 215us all-DVE), so the Pool fracs default to 0.
Q_ACT = int(_os.environ.get("GAT_Q_ACT", "7"))        # k's of q-mul on ACT (Copy+scale)
E_L1_POOL = float(_os.environ.get("GAT_E_L1_POOL", "0"))  # frac of e-tree L1 on GPSIMD
H_L1_POOL = float(_os.environ.get("GAT_H_L1_POOL", "0"))  # frac of h-tree L1 on GPSIMD
BUFS = int(_os.environ.get("GAT_BUFS", "3"))          # compute pool buffering


def _tree_steps(s):
    """Halving steps for an in-place prefix tree-sum of s elements:
    out[0:h] += in[keep:s], leaving keep = s - h live. Ends at s == 2."""
    steps = []
    while s > 2:
        h = s // 2
        steps.append((h, s - h, s))
        s = s - h
    return steps


def _tree_steps_even(s):
    """Like _tree_steps but h/keep stay even: the DVE 2x (fp16) mode
    requires 4B-aligned accesses, so fp16 slices along the innermost k
    axis must start at even element offsets."""
    assert s % 2 == 0
    steps = []
    while s > 2:
        h = (s // 2) & ~1
        steps.append((h, s - h, s))
        s = s - h
    return steps


def build(n_tiles: int, repeats: int = 1, mode: str = "full", klist=None):
    if klist is None:
        klist = [K] * (n_tiles // J)
    assert len(klist) == n_tiles // J
    sumkf = sum(2 * J * kp * D for kp in klist)  # ent+wr combined elems/row-p

    nc = bacc.Bacc("TRN2", target_bir_lowering=False, debug=False,
                   num_devices=N_CORES)

    cw_d = nc.dram_tensor("cw", [P * sumkf], F16, kind="ExternalInput")
    itemb_d = nc.dram_tensor("itemb", [P, n_tiles * D], F16, kind="ExternalInput")
    wt_d = nc.dram_tensor("wt", [D, D], F16, kind="ExternalInput")   # W_out.T
    ident_d = nc.dram_tensor("ident", [P, P], F16, kind="ExternalInput")
    out_d = nc.dram_tensor("out", [P, n_tiles * D], F16, kind="ExternalOutput")

    kmax = max(klist)

    with tile.TileContext(nc) as tc, ExitStack() as ctx:
        # bufs sized to each tile's pipeline span (stage of production
        # through stage of last use, see body_pairs)
        const = ctx.enter_context(tc.tile_pool(name="const", bufs=1))
        cwp = ctx.enter_context(tc.tile_pool(name="cwp", bufs=BUFS))
        wep = ctx.enter_context(tc.tile_pool(name="wep", bufs=BUFS + 2))
        qp = ctx.enter_context(tc.tile_pool(name="qp", bufs=BUFS))
        esp = ctx.enter_context(tc.tile_pool(name="esp", bufs=2))
        small = ctx.enter_context(tc.tile_pool(name="small", bufs=BUFS))
        psum = ctx.enter_context(tc.tile_pool(name="psum", bufs=4, space="PSUM"))

        itemb = const.tile([P, n_tiles * D], F16)
        wt = const.tile([D, D], F16)
        ident = const.tile([P, P], F16)
        out_all = const.tile([P, n_tiles * D], F16)
        nc.sync.dma_start(itemb[:], itemb_d[:])
        nc.sync.dma_start(wt[:], wt_d[:])
        nc.sync.dma_start(ident[:], ident_d[:])

        def tile_loop():
            body_pairs(nc, n_tiles, klist, kmax, cw_d, out_d, itemb, wt,
                       ident, out_all, cwp, wep, qp, esp, small, psum, mode)

        if repeats > 1:
            with tc.For_i(0, repeats, 1):
                tile_loop()
        else:
            tile_loop()

    nc.compile()
    return nc


def body_pairs(nc, n_tiles, klist, kmax, cw_d, out_d, itemb, wt, ident,
               out_all, cwp, wep, qp, esp, small, psum, mode):
    """Software-pipelined pair loop.

    The kernel is latency-bound, not throughput-bound: each pair's chain
    (DMA -> mul -> e-tree -> softmax -> q -> h-tree -> PE -> out) is ~10us
    of serially-dependent work, and measurements show the tile scheduler
    does NOT overlap consecutive pairs on its own (engines run their
    streams in order, so a mid-pair stall blocks the next pair's ops).
    Emitting the stages interleaved - at tick t, stage s handles pair
    t - s, deepest stage first - keeps ~7 pairs in flight so every
    engine always has ready work from SOME pair.
    """
    AF = mybir.ActivationFunctionType
    AL = mybir.AluOpType
    n_pairs = n_tiles // J

    offs = []
    off = 0
    for kp in klist:
        offs.append(off)
        off += 2 * J * P * kp * D

    st = [None] * n_pairs

    def s0_load(pg):
        kp = klist[pg]
        kf = kp * D
        blk = 2 * J * P * kf
        # one interleaved DMA: [:, :J*kf] = ent, [:, J*kf:] = wr
        cw = cwp.tile([P, 2 * J * kmax * D], F16, tag="cw")
        nc.sync.dma_start(
            cw[:, :2 * J * kf].rearrange("p (t j f) -> p t j f", t=2, j=J),
            cw_d[offs[pg]:offs[pg] + blk].rearrange(
                "(t j p f) -> p t j f", t=2, j=J, p=P))
        st[pg] = {"cw": cw, "kp": kp, "kf": kf}

    def s1_wemul(pg):
        d = st[pg]
        kp, kf, cw = d["kp"], d["kf"], d["cw"]
        we = wep.tile([P, J * kmax * D], F16, tag="we")
        nc.vector.tensor_mul(we[:, :J * kf], cw[:, :J * kf],
                             cw[:, J * kf:2 * J * kf])
        d["we"] = we
        d["we4"] = we[:, :J * kf].rearrange("p (j d k) -> p j d k",
                                            j=J, k=kp)

    def s2_elogits(pg):
        d = st[pg]
        kp, kf, we = d["kp"], d["kf"], d["we"]
        # e_{j,k} = sum_d We[j, d, k]: fp16 tree over d. All slices are
        # d-ranges of the (j, d, k) layout = contiguous runs, so the APs
        # merge and the DVE runs at full fp16 rate (short innermost runs
        # cost ~7 cycles of AP-walk per run and must be avoided).
        wej = we[:, :J * kf].rearrange("p (j f) -> p j f", j=J)
        e = small.tile([P, J * kmax], F32, tag="e")
        es = esp.tile([P, J * 50 * kmax], F16, tag="es")
        ese = es[:, :J * 50 * kp].rearrange("p (j f) -> p j f", j=J)
        nc.vector.tensor_add(ese, wej[:, :, 0:50 * kp],
                             wej[:, :, 50 * kp:100 * kp])
        for h, keep, s in _tree_steps(50):
            nc.vector.tensor_add(ese[:, :, 0:h * kp],
                                 ese[:, :, 0:h * kp],
                                 ese[:, :, keep * kp:s * kp])
        nc.vector.tensor_add(
            e[:, :J * kp].rearrange("p (j k) -> p j k", j=J),
            ese[:, :, 0:kp], ese[:, :, kp:2 * kp])
        d["e"] = e

    def s2b_exp(pg):
        d = st[pg]
        kp, e = d["kp"], d["e"]
        # leaky relu (DVE), exp (ACT) - its own stage so the exp has a
        # full tick before s3 consumes it
        elr = small.tile([P, J * kmax], F32, tag="elr")
        nc.vector.scalar_tensor_tensor(elr[:, :J * kp], e[:, :J * kp],
                                       ALPHA, e[:, :J * kp],
                                       op0=AL.mult, op1=AL.max)
        p = small.tile([P, J * kmax], F32, tag="p")
        nc.scalar.activation(p[:, :J * kp], elr[:, :J * kp], AF.Exp)
        d["p"] = p

    def s3_attq(pg):
        d = st[pg]
        kp, kf, p, we4 = d["kp"], d["kf"], d["p"], d["we4"]
        # normalize attention before weighting: a = p / sum_k p (in [0,1],
        # fp16-safe; raw exp(e) ~ 1e17 is not)
        sumexp = small.tile([P, J], F32, tag="sumexp")
        nc.vector.tensor_reduce(
            sumexp[:], p[:, :J * kp].rearrange("p (j k) -> p j k", j=J),
            axis=mybir.AxisListType.X, op=AL.add)
        rs = small.tile([P, J], F32, tag="rs")
        nc.vector.reciprocal(rs[:], sumexp[:])
        ph = small.tile([P, J * kmax], F16, tag="ph")
        for j in range(J):
            jsl = slice(j * kp, (j + 1) * kp)
            nc.vector.scalar_tensor_tensor(ph[:, jsl], p[:, jsl],
                                           rs[:, j:j + 1], p[:, jsl],
                                           op0=AL.mult, op1=AL.bypass)
        # q = We * a  (DVE fp16 2x: broadcast over d keeps k innermost)
        q = qp.tile([P, J * kmax * D], F16, tag="q")
        q4 = q[:, :J * kf].rearrange("p (j d k) -> p j d k", j=J, k=kp)
        ph4 = (ph[:, :J * kp].rearrange("p (j k) -> p j k", j=J)
               .unsqueeze(2).broadcast_to([P, J, D, kp]))
        nc.vector.tensor_mul(q4, we4, ph4)
        d["q4"] = q4

    def s4_htrans(pg):
        d = st[pg]
        kp, q4 = d["kp"], d["q4"]
        # h'^T[j] = sum_k q[:, j, :, k]^T, accumulated exactly in fp32
        # PSUM by one PE matmul per k (lhsT = strided k-slice of q,
        # rhs = identity -> transpose; start=False accumulates). This
        # replaces the DVE k-tree, whose short innermost runs made it
        # ~5x slower than its element count suggests.
        d["htp"] = []
        for j in range(J):
            ht_ps = psum.tile([D, P], F32, tag="htp")
            for k in range(kp):
                nc.tensor.matmul(ht_ps[:], q4[:, j:j + 1, :, k:k + 1],
                                 ident[:], start=(k == 0),
                                 stop=(k == kp - 1))
            d["htp"].append(ht_ps)

    def s5_htcopy(pg):
        d = st[pg]
        # h'^T PSUM -> SBUF fp16 (ACT); own stage so the x-matmul a tick
        # later never waits on ACT
        d["ht"] = []
        for j in range(J):
            ht = small.tile([D, P], F16, tag="ht")
            nc.scalar.copy(ht[:], d["htp"][j][:])
            d["ht"].append(ht)

    def s5b_mm(pg):
        d = st[pg]
        d["xps"] = []
        for j in range(J):
            x_ps = psum.tile([P, D], F32, tag="xps")
            nc.tensor.matmul(x_ps[:], d["ht"][j][:], wt[:],
                             start=True, stop=True)
            d["xps"].append(x_ps)

    def s6_out(pg):
        d = st[pg]
        for j in range(J):
            t = pg * J + j
            # out = x + (item + b)  (DVE epilogue)
            nc.vector.tensor_add(out_all[:, t * D:(t + 1) * D],
                                 d["xps"][j],
                                 itemb[:, t * D:(t + 1) * D])
        if (pg + 1) % (STORE_CHUNK // J) == 0:
            csl = slice((pg + 1 - STORE_CHUNK // J) * J * D,
                        (pg + 1) * J * D)
            nc.sync.dma_start(out_d[:, csl], out_all[:, csl])
        st[pg] = None

    if mode == "dma":
        for pg in range(n_pairs):
            s0_load(pg)
            kf = st[pg]["kf"]
            for j in range(J):
                t = pg * J + j
                nc.vector.tensor_copy(out_all[:, t * D:(t + 1) * D],
                                      st[pg]["cw"][:, j * kf:j * kf + D])
            if (pg + 1) % (STORE_CHUNK // J) == 0:
                csl = slice((pg + 1 - STORE_CHUNK // J) * J * D,
                            (pg + 1) * J * D)
                nc.sync.dma_start(out_d[:, csl], out_all[:, csl])
    else:
        stages = [s0_load, s1_wemul, s2_elogits, s2b_exp, s3_attq,
                  s4_htrans, s5_htcopy, s5b_mm, s6_out]
        NS = len(stages)
        for t in range(n_pairs + NS - 1):
            for s in reversed(range(NS)):
                pg = t - s
                if 0 <= pg < n_pairs:
                    stages[s](pg)

    rem = n_pairs % (STORE_CHUNK // J)
    if rem:
        csl = slice((n_pairs - rem) * J * D, n_pairs * J * D)
        nc.sync.dma_start(out_d[:, csl], out_all[:, csl])


def _shard_host(item_embs, entity_embs, w_r, adj, W_out, b_out, n_tiles):
    """Sort rows by active-neighbor count, pack active k's first, poison the
    masked tail slots, transpose each row to [D, kp] (k innermost), fp16,
    and interleave ent|wr into one per-core buffer. Pairs striped across
    cores as v1. Returns (in_maps, klist, order)."""
    rows = n_tiles * P
    n_pad = N_CORES * rows
    n_pairs = n_tiles // J

    ent = np.asarray(entity_embs, np.float32).reshape(N, K, D)
    wr = np.asarray(w_r, np.float32).reshape(N, K, D)
    adjf = np.asarray(adj).astype(np.float32)
    itemb = np.asarray(item_embs, np.float32) + np.asarray(b_out, np.float32)

    pad = n_pad - N
    ent = np.pad(ent, ((0, pad), (0, 0), (0, 0)))
    wr = np.pad(wr, ((0, pad), (0, 0), (0, 0)))
    # padding rows: one active zero neighbor -> e=0, sumexp=1 (count 1
    # sorts them to the sparse end); their output rows are discarded.
    adjp = np.pad(adjf, ((0, pad), (0, 0)))
    adjp[N:, 0] = 1.0
    itemb = np.pad(itemb, ((0, pad), (0, 0)))

    counts = adjp.sum(1).astype(np.int64)
    order = np.argsort(counts, kind="stable")

    # round packed K up to even: the DVE fp16 2x mode needs 4B alignment,
    # so odd kp would misalign every (j, d) row and all k-axis tree slices
    pair_k = counts[order].reshape(-1, J * P).max(1)
    klist = []
    for j in range(n_pairs):
        kp = max(2, int(pair_k[8 * j: 8 * j + 8].max()))
        klist.append(kp + (kp & 1))

    ai_full = np.argsort(1.0 - adjp, axis=1, kind="stable")  # active first

    wt = np.ascontiguousarray(np.asarray(W_out, np.float32).T).astype(np.float16)
    ident = np.eye(P, dtype=np.float16)

    in_maps = []
    for c in range(N_CORES):
        cw_parts = []
        it_sw = np.empty((P, n_tiles * D), np.float16)
        for j in range(n_pairs):
            g = 8 * j + c
            rsel = order[g * J * P:(g + 1) * J * P]
            kp = klist[j]
            ai = ai_full[rsel, :kp]
            cnt = counts[rsel]                               # [256]
            eg = np.take_along_axis(ent[rsel], ai[:, :, None], 1)  # [256,kp,D]
            wg = np.take_along_axis(wr[rsel], ai[:, :, None], 1)
            # poison masked tail slots: We[d0] = -POISON^2, rest 0 -> exp=0
            mask = np.arange(kp)[None, :] >= cnt[:, None]    # [256, kp]
            eg[mask] = 0.0
            wg[mask] = 0.0
            eg[:, :, 0][mask] = -POISON
            wg[:, :, 0][mask] = POISON
            # k-innermost: [256, kp, D] -> [256, D, kp]; fp16
            eg = eg.transpose(0, 2, 1).astype(np.float16)
            wg = wg.transpose(0, 2, 1).astype(np.float16)
            cw_parts.append(eg.ravel())
            cw_parts.append(wg.ravel())
            it = itemb[rsel].reshape(J, P, D).transpose(1, 0, 2)
            it_sw[:, j * J * D:(j + 1) * J * D] = \
                it.reshape(P, J * D).astype(np.float16)
        in_maps.append({
            "cw": np.concatenate(cw_parts),
            "itemb": it_sw,
            "wt": wt,
            "ident": ident,
        })
    return in_maps, klist, order


def _unshard_host(results, n_tiles, order):
    n_pairs = n_tiles // J
    res_sorted = np.empty((N_CORES * n_tiles * P, D), np.float32)
    for c in range(N_CORES):
        o = results[c]["out"].astype(np.float32)  # [P, n_tiles * D] fp16
        for j in range(n_pairs):
            g = 8 * j + c
            blk = (o[:, j * J * D:(j + 1) * J * D]
                   .reshape(P, J, D).transpose(1, 0, 2).reshape(J * P, D))
            res_sorted[g * J * P:(g + 1) * J * P] = blk
    out = np.empty_like(res_sorted)
    out[order] = res_sorted
    return out[:N]


def kernel(item_embs, entity_embs, w_r, adj, W_out, b_out):
    from concourse.bass_utils import run_bass_kernel_spmd

    in_maps, klist, order = _shard_host(item_embs, entity_embs, w_r, adj,
                                        W_out, b_out, _N_TILES_FULL)
    nc = build(_N_TILES_FULL, klist=klist)
    res = run_bass_kernel_spmd(nc, in_maps, core_ids=list(range(N_CORES)))
    return _unshard_host(res.results, _N_TILES_FULL, order).astype(np.float32)
